# revision 1
# baseline (speedup 1.0000x reference)
"""Trainium2 Bass kernel for ControllableNCA forward step.

Data-parallel over 8 NeuronCores: batch 64 -> 8 images per core.

Per-core structure (vs the original slab kernel: ~25% less HBM traffic,
~3x fewer DMA instructions, 2x fewer PSUM drain ops, no PE drain-stalls):
  - row layout [h=128 partitions, (c, w) free] for all elementwise work;
    masks broadcast over channels via free-dim step-0 APs; vertical
    maxpool shifts via small SBUF->SBUF DMAs of the live-channel plane.
  - conv as 3 accumulating dx-tap matmuls (K=60 over a dy-stacked
    [60, pos] operand) + w2 matmul, in 512-column chunks over a whole
    half-image (flat 130-pitch padded position space, 17 chunks/half).
  - the dy-stack S is built once per half: one fat DRAM read of the
    xg scratch into group 0 (66 plane rows incl. the dy=2 halo), then
    two fat SBUF->SBUF shifted copies for groups 1/2 — the conv reads
    each xg byte from DRAM once (the slab kernel read it 3x).
  - chunk pairs share 2-bank PSUM tiles so ReLU+bias (ACT) and out
    drains (DVE/ACT alternating) run once per 1024 columns; taps run
    dx-major (fewer weight switches) and each pair's w2 matmuls are
    deferred two pairs behind the taps so the PE never idles waiting
    on the ACT ReLU drain.
  - out returns to row layout via one DRAM bounce per half.
"""
import sys

sys.path.insert(0, "/opt/trn_rl_repo")

import dataclasses

import numpy as np

import concourse.bacc as bacc
import concourse.tile as tile
from concourse import mybir
from concourse import bass_utils

NCORES = 8
B = 64
BL = B // NCORES
C = 20
H = 128
W = 128
WP = 130          # padded row pitch (1 left + 1 right pad col)
CH = 3 * C        # stacked contraction dim (c, dy)
HID = 128
HALF = 64         # rows per half-image
NCHUNK = 17       # 512-chunks per half (17*512 = 8704 >= 64*130)
SW = NCHUNK * 512 + 2        # read extent (8706): chunks + dx lookahead
SWX = SW + 2 * WP            # S width incl. zero slack the fat copies shift in
CHUNK = 512
LIVING = 3
ALIVE_TH = 0.1
FIRE_RATE = 0.5
NEG = -1.0e9

F32 = mybir.dt.float32
FR = mybir.dt.float32r

_cache = {}
_last_in_maps = None


def _row_src(img_ap, c, h, w, h0=0, nh=H):
    """AP over DRAM image [c, h, w] iterated in (h, c, w) order."""
    return dataclasses.replace(
        img_ap, ap=[[w, nh], [h * w, c], [1, w]], offset=img_ap.offset + h0 * w
    )


def _bcast_c(mask_ap, nc_count, w):
    """[128, W] mask viewed as [128, (0,C), W] to broadcast over channels."""
    return dataclasses.replace(
        mask_ap, ap=[list(mask_ap.ap[0]), [0, nc_count], [1, w]]
    )


def _free_ap(sl, dims, extra_off=0):
    """Replace the free dims of a single-partition slice with `dims`."""
    return dataclasses.replace(
        sl, ap=[list(sl.ap[0])] + dims, offset=sl.offset + extra_off
    )


def _build_program():
    nc = bacc.Bacc("TRN2", debug=False, num_devices=NCORES)

    x_d = nc.dram_tensor("x", [BL, C, H, W], F32, kind="ExternalInput").ap()
    g_d = nc.dram_tensor("g", [BL, C, H, W], F32, kind="ExternalInput").ap()
    r_d = nc.dram_tensor("r", [BL, H, W], F32, kind="ExternalInput").ap()
    wst_d = nc.dram_tensor("wst", [CH, 3, HID], FR, kind="ExternalInput").ap()
    w2t_d = nc.dram_tensor("w2t", [HID, C], FR, kind="ExternalInput").ap()
    b1_d = nc.dram_tensor("b1", [HID, 1], F32, kind="ExternalInput").ap()
    xg_s = nc.dram_tensor("xg_s", [BL, C, H, WP], F32, kind="Internal").ap()
    out_s = nc.dram_tensor("out_s", [BL, C, H, W], F32, kind="Internal").ap()
    y_d = nc.dram_tensor("y", [BL, C, H, W], F32, kind="ExternalOutput").ap()

    with tile.TileContext(nc) as tc:
        with (
            tc.tile_pool(name="singles", bufs=1) as singles,
            tc.tile_pool(name="xrows", bufs=2) as xrows,
            tc.tile_pool(name="grows", bufs=2) as grows,
            tc.tile_pool(name="xgp", bufs=1) as xgp,
            tc.tile_pool(name="maskp", bufs=2) as maskp,
            tc.tile_pool(name="shiftp", bufs=1) as shiftp,
            tc.tile_pool(name="stackp", bufs=2) as stackp,
            tc.tile_pool(name="outcm", bufs=1) as outcmp,
            tc.tile_pool(name="orow", bufs=1) as orowp,
            tc.tile_pool(name="xnpp", bufs=1) as xnpp,
            tc.tile_pool(name="hbuf", bufs=4) as hbuf,
            tc.tile_pool(name="psA", bufs=2, space="PSUM") as psA,
            tc.tile_pool(name="psB", bufs=2, space="PSUM") as psB,
        ):
            wst = singles.tile([CH, 3, HID], FR)
            nc.sync.dma_start(out=wst, in_=wst_d)
            w2t = singles.tile([HID, C], FR)
            nc.sync.dma_start(out=w2t, in_=w2t_d)
            b1 = singles.tile([HID, 1], F32)
            nc.sync.dma_start(out=b1, in_=b1_d)

            for b in range(BL):
                # ---- loads ------------------------------------------------
                x_row = xrows.tile([H, C, W], F32)
                nc.sync.dma_start(out=x_row, in_=_row_src(x_d[b], C, H, W))
                g_row = grows.tile([H, C, W], F32)
                nc.sync.dma_start(out=g_row, in_=_row_src(g_d[b], C, H, W))
                r_t = maskp.tile([H, W], F32, tag="rt")
                nc.sync.dma_start(out=r_t, in_=r_d[b])

                # ---- pre life mask: maxpool3(x[:,3]) > 0.1 ---------------
                pu = shiftp.tile([H, W], F32, tag="pu")
                nc.vector.memset(pu, NEG)
                nc.sync.dma_start(out=pu[0 : H - 1, :], in_=x_row[1:H, LIVING, :])
                pd = shiftp.tile([H, W], F32, tag="pd")
                nc.vector.memset(pd[0:1, :], NEG)
                nc.sync.dma_start(out=pd[1:H, :], in_=x_row[0 : H - 1, LIVING, :])
                vm = shiftp.tile([H, W + 2], F32, tag="vm")
                nc.vector.memset(vm[:, 0:1], NEG)
                nc.vector.memset(vm[:, W + 1 : W + 2], NEG)
                nc.vector.tensor_max(
                    out=vm[:, 1 : W + 1], in0=x_row[:, LIVING, :], in1=pu
                )
                nc.vector.tensor_max(out=vm[:, 1 : W + 1], in0=vm[:, 1 : W + 1], in1=pd)
                hm = shiftp.tile([H, W], F32, tag="hm")
                nc.vector.tensor_max(out=hm, in0=vm[:, 0:W], in1=vm[:, 1 : W + 1])
                nc.vector.tensor_max(out=hm, in0=hm, in1=vm[:, 2 : W + 2])
                mpre = maskp.tile([H, W], F32, tag="mpre")
                nc.vector.tensor_scalar(
                    out=mpre, in0=hm, scalar1=ALIVE_TH, scalar2=None,
                    op0=mybir.AluOpType.is_gt,
                )

                # ---- rand mask -------------------------------------------
                rm = maskp.tile([H, W], F32, tag="rm")
                nc.vector.tensor_scalar(
                    out=rm, in0=r_t, scalar1=FIRE_RATE, scalar2=None,
                    op0=mybir.AluOpType.is_lt,
                )

                # ---- xg = x + g * mpre (row layout, padded cols) ---------
                xg_row = xgp.tile([H, C, WP], F32)
                nc.vector.memset(xg_row[:, :, 0:1], 0.0)
                nc.vector.memset(xg_row[:, :, W + 1 : W + 2], 0.0)
                xgi = xg_row[:, :, 1 : W + 1]
                nc.gpsimd.tensor_tensor(
                    out=xgi, in0=g_row, in1=_bcast_c(mpre, C, W),
                    op=mybir.AluOpType.mult,
                )
                nc.gpsimd.tensor_tensor(
                    out=xgi, in0=xgi, in1=x_row, op=mybir.AluOpType.add
                )
                # bounce xg to DRAM scratch in [c, h, wp] layout
                nc.sync.dma_start(out=_row_src(xg_s[b], C, H, WP), in_=xg_row)

                out_cm = outcmp.tile([C, SW], F32)
                out_row = orowp.tile([H, C, W], F32)

                # ---- conv, half-image at a time --------------------------
                for hf in range(2):
                    h0 = hf * HALF
                    S = stackp.tile([CH, SWX], FR)
                    Sf = S.bitcast(F32)
                    # group 0 holds padded-plane rows h0 .. h0+64
                    # (plane row gg = xg row gg-1; row 0 is the zero top pad)
                    if hf == 0:
                        nc.vector.memset(Sf[0:C, 0:WP], 0.0)
                        src_h0, nrow, d0 = 0, HALF + 1, WP
                    else:
                        src_h0, nrow, d0 = h0 - 1, HALF + 1, 0
                    src = dataclasses.replace(
                        xg_s[b],
                        ap=[[H * WP, C], [WP, nrow], [1, WP]],
                        offset=xg_s[b].offset + src_h0 * WP,
                    )
                    nc.sync.dma_start(
                        out=Sf[0:C, d0 : d0 + nrow * WP], in_=src
                    )
                    nc.vector.memset(Sf[0:C, d0 + nrow * WP : SWX], 0.0)
                    # groups 1, 2: fat shifted copies of group 0 (the source
                    # zero tail shifts in, so no partition-offset memsets)
                    nc.sync.dma_start(
                        out=Sf[C : 2 * C, 0 : SWX - WP], in_=Sf[0:C, WP:SWX]
                    )
                    nc.sync.dma_start(
                        out=Sf[2 * C : 3 * C, 0 : SW],
                        in_=Sf[0:C, 2 * WP : 2 * WP + SW],
                    )

                    # ---- chunk loop (pairs share 2-bank PSUM tiles) ------
                    # dx-major tap order halves LDWEIGHTS switches; the w2
                    # matmuls for pair p are deferred until after pair p+1's
                    # taps, so the PE never waits on the ACT ReLU drain.
                    def _emit_w2(hs_, n_, fb_, kp_):
                        p2 = psB.tile([C, 2 * CHUNK], F32, tag="p2")
                        for j in range(n_ // CHUNK):
                            nc.tensor.matmul(
                                p2[:, j * CHUNK : (j + 1) * CHUNK],
                                w2t,
                                hs_[:, j * CHUNK : (j + 1) * CHUNK],
                                start=True, stop=True,
                            )
                        if kp_ % 3 == 2:
                            nc.scalar.activation(
                                out=out_cm[:, fb_ : fb_ + n_], in_=p2[:, 0:n_],
                                func=mybir.ActivationFunctionType.Copy,
                                bias=0.0, scale=1.0,
                            )
                        else:
                            nc.vector.tensor_copy(
                                out=out_cm[:, fb_ : fb_ + n_], in_=p2[:, 0:n_]
                            )

                    pending = []
                    for kp in range((NCHUNK + 1) // 2):
                        k0 = 2 * kp
                        npair = min(2, NCHUNK - k0)
                        n = npair * CHUNK
                        ps = psA.tile([HID, 2 * CHUNK], F32, tag="ps")
                        for dx in range(3):
                            for j in range(npair):
                                f0 = (k0 + j) * CHUNK
                                nc.tensor.matmul(
                                    ps[:, j * CHUNK : (j + 1) * CHUNK],
                                    wst[:, dx, :],
                                    S[:, f0 + dx : f0 + dx + CHUNK],
                                    start=(dx == 0),
                                    stop=(dx == 2),
                                )
                        hs = hbuf.tile([HID, 2 * CHUNK], FR, tag="hs")
                        nc.scalar.activation(
                            out=hs[:, 0:n], in_=ps[:, 0:n],
                            func=mybir.ActivationFunctionType.Relu,
                            bias=b1, scale=1.0,
                        )
                        pending.append((hs, n, k0 * CHUNK, kp))
                        if len(pending) > 2:
                            _emit_w2(*pending.pop(0))
                    for args in pending:
                        _emit_w2(*args)

                    # ---- out_cm -> DRAM scratch (valid cols only) --------
                    osrc = dataclasses.replace(
                        out_cm[:, :], ap=[[SW, C], [WP, HALF], [1, W]]
                    )
                    odst = dataclasses.replace(
                        out_s[b],
                        ap=[[H * W, C], [W, HALF], [1, W]],
                        offset=out_s[b].offset + h0 * W,
                    )
                    nc.sync.dma_start(out=odst, in_=osrc)

                # ---- x_new_pre = x + rm * out ----------------------------
                nc.sync.dma_start(out=out_row, in_=_row_src(out_s[b], C, H, W))
                xnp = xnpp.tile([H, C, W], F32)
                nc.gpsimd.tensor_tensor(
                    out=xnp, in0=out_row, in1=_bcast_c(rm, C, W),
                    op=mybir.AluOpType.mult,
                )
                nc.gpsimd.tensor_tensor(
                    out=xnp, in0=xnp, in1=x_row, op=mybir.AluOpType.add
                )

                # ---- post life mask on x_new_pre[:, 3, :] ----------------
                qu = shiftp.tile([H, W], F32, tag="qu")
                nc.vector.memset(qu, NEG)
                nc.sync.dma_start(out=qu[0 : H - 1, :], in_=xnp[1:H, LIVING, :])
                qd = shiftp.tile([H, W], F32, tag="qd")
                nc.vector.memset(qd[0:1, :], NEG)
                nc.sync.dma_start(out=qd[1:H, :], in_=xnp[0 : H - 1, LIVING, :])
                vm2 = shiftp.tile([H, W + 2], F32, tag="vm2")
                nc.vector.memset(vm2[:, 0:1], NEG)
                nc.vector.memset(vm2[:, W + 1 : W + 2], NEG)
                nc.vector.tensor_max(
                    out=vm2[:, 1 : W + 1], in0=xnp[:, LIVING, :], in1=qu
                )
                nc.vector.tensor_max(
                    out=vm2[:, 1 : W + 1], in0=vm2[:, 1 : W + 1], in1=qd
                )
                hm2 = shiftp.tile([H, W], F32, tag="hm2")
                nc.vector.tensor_max(out=hm2, in0=vm2[:, 0:W], in1=vm2[:, 1 : W + 1])
                nc.vector.tensor_max(out=hm2, in0=hm2, in1=vm2[:, 2 : W + 2])
                # life = (maxpool > th) * mpre
                life = maskp.tile([H, W], F32, tag="life")
                nc.vector.scalar_tensor_tensor(
                    out=life, in0=hm2, scalar=ALIVE_TH, in1=mpre,
                    op0=mybir.AluOpType.is_gt, op1=mybir.AluOpType.mult,
                )

                # ---- y = clip(x_new_pre * life, -10, 10) -----------------
                nc.gpsimd.tensor_tensor(
                    out=xnp, in0=xnp, in1=_bcast_c(life, C, W),
                    op=mybir.AluOpType.mult,
                )
                nc.vector.tensor_scalar(
                    out=xnp, in0=xnp, scalar1=-10.0, scalar2=10.0,
                    op0=mybir.AluOpType.max, op1=mybir.AluOpType.min,
                )
                nc.sync.dma_start(out=_row_src(y_d[b], C, H, W), in_=xnp)

    nc.compile()
    return nc


def kernel(x, goal_encoding, rand_uniform, pw, w1, b1, w2):
    if "nc" not in _cache:
        _cache["nc"] = _build_program()
    nc = _cache["nc"]

    x = np.ascontiguousarray(x, dtype=np.float32)
    goal_encoding = np.ascontiguousarray(goal_encoding, dtype=np.float32)
    rand_uniform = np.ascontiguousarray(rand_uniform, dtype=np.float32)

    # Fold depthwise perception conv + w1 into one dense 3x3 conv:
    # W_eff[o, c, dy, dx] = sum_l w1[o, 3c+l] * pw[3c+l, 0, dy, dx]
    pw_ = np.asarray(pw, dtype=np.float64).reshape(C, 3, 3, 3)  # [c, l, dy, dx]
    w1_ = np.asarray(w1, dtype=np.float64).reshape(HID, C, 3)  # [o, c, l]
    weff = np.einsum("ocl,cldx->ocdx", w1_, pw_)  # [o, c, dy, dx]
    wst = np.transpose(weff, (2, 1, 3, 0))  # [dy, c, dx, o]
    wst = np.ascontiguousarray(wst.reshape(3 * C, 3, HID), dtype=np.float32)
    w2t = np.ascontiguousarray(np.asarray(w2, dtype=np.float32).T)  # [128, 20]
    b1v = np.asarray(b1, dtype=np.float32).reshape(HID, 1)

    in_maps = []
    for i in range(NCORES):
        sl = slice(i * BL, (i + 1) * BL)
        in_maps.append(
            {
                "x": x[sl],
                "g": goal_encoding[sl],
                "r": np.ascontiguousarray(rand_uniform[sl].reshape(BL, H, W)),
                "wst": wst,
                "w2t": w2t,
                "b1": b1v,
            }
        )

    global _last_in_maps
    _last_in_maps = in_maps
    res = bass_utils.run_bass_kernel_spmd(nc, in_maps, core_ids=list(range(NCORES)))
    out = np.concatenate([res.results[i]["y"] for i in range(NCORES)], axis=0)
    return out

